# revision 6
# baseline (speedup 1.0000x reference)
"""Bass/Tile kernel for multi-head self-attention on 8 TRN2 NeuronCores.

Problem: B=16, S=1024, D=768, H=12, head_dim=64, fp32 in/out.
Strategy: data parallel over batch (2 batch items per core, no collectives).

v2: software-pipelined schedule. The 2 batches x 6 head-pairs x 2 i-chunks
form 24 "phases"; phase N's P@V matmuls run interleaved with phase N+1's
QK^T+exp (probsT is a 12-deep ring of per-jt tiles), so ScalarE's exp
stream (the second-busiest engine) never waits on TensorE bursts and vice
versa. All per-batch tensors are double-buffered (qT/kT as chunk-granular
rings) so batch 1's transposes/projections fill batch 0's PE gaps, and
batch 0's output projection runs inside batch 1's pair loop. Aux work
(weight transposes, x transposes, projections, output proj) is emitted as
small quanta pumped one-per-jt-slot between QK/PV emissions.

Per-core layout (bf16 matmul operands, fp32 accumulation):
  - x PE-transposed to xT [c, t]; weights pre-transposed to wT [c_in,
    c_out] (wv/wo share one buffer: wo replaces wv after the V proj).
  - qT, kT feature-major per-chunk ring tiles; v token-major with a ones
    column per head (v_aug) so P@V also yields softmax denominators.
  - scoresT [j, i] via 2-head row-packed matmuls (K=64 at partitions
    0/64 run concurrently); exp on ScalarE from PSUM with 1/8 scale and
    +ln16 bias folded in (cancels between numerator and denominator).
  - P@V accumulates over 8 key tiles into [65, 512] PSUM; row 64 holds
    the denominators, DMA'd via a staging tile to rsum.
  - Normalization: reciprocal of rsum, broadcast across partitions via a
    one-hot selector matmul, DVE multiply into attn_T in place.
  - Output projection from attn_T, natural [t, o] tiles DMA'd to DRAM.
"""

import contextlib
import threading
from collections import deque

import numpy as np

import concourse.bass as bass
import concourse.tile as tile
from concourse import bacc, mybir
from concourse.bass_utils import run_bass_kernel_spmd
from concourse.masks import make_identity

N_CORES = 8
B, S, D = 16, 1024, 768
H, HD = 12, 64
BPC = B // N_CORES  # batch items per core

P = 128
CC = D // P          # 6 feature chunks of 128
TN = 512             # matmul moving free dim
NT = S // TN         # 2 token chunks of 512
TT = S // P          # 8 token tiles of 128
JT = S // P          # 8 key tiles of 128

F32 = mybir.dt.float32
BF16 = mybir.dt.bfloat16
F32R = mybir.dt.float32r

AF = mybir.ActivationFunctionType
ALU = mybir.AluOpType


def build_kernel(tc: "tile.TileContext", outs, ins):
    nc = tc.nc
    x_d = ins["x"]
    out_d = outs["out"]

    ctx = contextlib.ExitStack()
    with ctx:
        const = ctx.enter_context(tc.tile_pool(name="const", bufs=1))
        wpool = ctx.enter_context(tc.tile_pool(name="wts", bufs=1))
        iop = ctx.enter_context(tc.tile_pool(name="iop", bufs=3))
        big = ctx.enter_context(tc.tile_pool(name="big", bufs=2))
        qkc = ctx.enter_context(tc.tile_pool(name="qkc", bufs=4))
        pbt = ctx.enter_context(tc.tile_pool(name="pbt", bufs=12))
        small = ctx.enter_context(tc.tile_pool(name="small", bufs=2))
        st64p = ctx.enter_context(tc.tile_pool(name="st64", bufs=4))
        psq = ctx.enter_context(tc.tile_pool(name="psq", bufs=2, space="PSUM"))
        ppv = ctx.enter_context(tc.tile_pool(name="ppv", bufs=2, space="PSUM"))
        pmm = ctx.enter_context(tc.tile_pool(name="pmm", bufs=2, space="PSUM"))

        # ---- one-time constants ----
        ident = const.tile([P, P], BF16)
        make_identity(nc, ident)

        ones_row = const.tile([1, P], F32)
        nc.vector.memset(ones_row, 1.0)
        ones_r = const.tile([1, P], F32R)
        nc.vector.tensor_copy(ones_r, ones_row)

        # head-selector matrix: sel[k, h*64+j] = (k == h); broadcasts
        # recip[h, :] across 64 output partitions with a K=12 matmul.
        sel_f = iop.tile([H, H * HD], F32, tag="natf", name="sel_f")
        nc.sync.dma_start(sel_f, ins["sel"])
        sel = const.tile([H, H * HD], BF16)
        nc.vector.tensor_copy(sel, sel_f)

        # +ln(16) folded into exp keeps probs away from denormals; the
        # factor cancels between P@V numerator and the denominators.
        ln16 = const.tile([P, 1], F32)
        nc.vector.memset(ln16, float(np.log(16.0)))

        # q/k biases laid out per-partition: [p, oc] = b[oc*128 + p]
        bq = const.tile([P, CC], F32)
        bk = const.tile([P, CC], F32)
        with nc.allow_non_contiguous_dma(reason="tiny bias load"):
            nc.sync.dma_start(bq, ins["wq_b"].rearrange("(oc p) -> p oc", p=P))
            nc.sync.dma_start(bk, ins["wk_b"].rearrange("(oc p) -> p oc", p=P))

        # v/out biases broadcast along partitions: [128, 768] via ones-matmul
        bias_bc = {}
        for name in ("wv_b", "wo_b"):
            brow_f = iop.tile([1, D], F32, name=f"{name}_rowf", tag="natf")
            nc.sync.dma_start(brow_f, ins[name][None, :])
            brow = iop.tile([1, D], F32R, name=f"{name}_row", tag="brow", bufs=1)
            nc.vector.tensor_copy(brow, brow_f)
            bc = const.tile([P, D], BF16, name=f"{name}_bc")
            for n0 in range(0, D, TN):
                nsz = min(TN, D - n0)
                pb = pmm.tile([P, TN], F32, tag="mm")
                nc.tensor.matmul(
                    pb[:, :nsz], ones_r, brow[:, n0 : n0 + nsz],
                    start=True, stop=True,
                )
                nc.vector.tensor_copy(bc[:, n0 : n0 + nsz], pb[:, :nsz])
            bias_bc[name] = bc

        # ---- persistent per-batch tensors (double-buffered by tag) ----
        xT = {}       # b -> [P, CC, S] bf16
        v_aug = {}    # b -> [P, TT, H, HD+1] bf16
        attn_T = {}   # b -> [P, CC, S] bf16
        rsum = {}     # b -> [H, S] f32
        recip_r = {}  # b -> [H, S] bf16
        qk_tiles = {}  # (qk, b, oc) -> [P, S] bf16 ring tile

        # wq/wk persist; wv and wo share one buffer (wo written after the
        # last V-projection read).
        wqT = wpool.tile([P, CC, D], BF16, name="wq_T")
        wkT = wpool.tile([P, CC, D], BF16, name="wk_T")
        wvoT = wpool.tile([P, CC, D], BF16, name="wvo_T")
        wT = {"wq_w": wqT, "wk_w": wkT, "wv_w": wvoT, "wo_w": wvoT}

        # ---------------- emission quanta ----------------
        def q_x_dma(b, tt):
            def go():
                xf = iop.tile([P, D], F32, tag="natf", name="xf")
                nc.sync.dma_start(xf, x_d[b, tt * P : (tt + 1) * P, :])
                xb = iop.tile([P, D], BF16, tag="natb", name="xb")
                nc.vector.tensor_copy(xb, xf)
                q_x_dma.stage[(b, tt)] = xb
            return go
        q_x_dma.stage = {}

        def q_x_transpose(b, tt):
            def go():
                if b not in xT:
                    xT[b] = big.tile([P, CC, S], BF16, tag="xT", name="xT")
                xb = q_x_dma.stage.pop((b, tt))
                ptr = pmm.tile([P, CC, P], BF16, tag="mm", name="xptr")
                for cc in range(CC):
                    nc.tensor.transpose(
                        ptr[:, cc], xb[:, cc * P : (cc + 1) * P], ident
                    )
                nc.vector.tensor_copy(xT[b][:, :, tt * P : (tt + 1) * P], ptr)
            return go

        def q_w_transpose(name, oc):
            # one oc chunk: DMA nat rows, cast, 6 transposes, evac to wT col
            def go():
                wt = wT[name]
                wnat_f = iop.tile([P, D], F32, tag="natf", name="wnat_f")
                nc.sync.dma_start(wnat_f, ins[name][oc * P : (oc + 1) * P, :])
                wnat = iop.tile([P, D], BF16, tag="natb", name="wnat")
                nc.vector.tensor_copy(wnat, wnat_f)
                ptr = pmm.tile([P, CC, P], BF16, tag="mm", name="wptr")
                for cc in range(CC):
                    nc.tensor.transpose(
                        ptr[:, cc], wnat[:, cc * P : (cc + 1) * P], ident
                    )
                nc.vector.tensor_copy(wt[:, :, oc * P : (oc + 1) * P], ptr)
            return go

        def q_qk_chunk_part(b, oc, qk, nt):
            # one 6-matmul chain: q or k chunk oc, token half nt
            def go():
                key = (qk, b, oc)
                if key not in qk_tiles:
                    qk_tiles[key] = qkc.tile(
                        [P, S], BF16, tag=f"{qk}Tc", name=f"{qk}Tc"
                    )
                dst = qk_tiles[key]
                wt = wT["wq_w" if qk == "q" else "wk_w"]
                bap = bq if qk == "q" else bk
                pq = pmm.tile([P, TN], F32, tag="mm", name="pq")
                for cc in range(CC):
                    nc.tensor.matmul(
                        pq,
                        wt[:, cc, oc * P : (oc + 1) * P],
                        xT[b][:, cc, nt * TN : (nt + 1) * TN],
                        start=(cc == 0),
                        stop=(cc == CC - 1),
                    )
                nc.vector.tensor_tensor(
                    dst[:, nt * TN : (nt + 1) * TN],
                    pq,
                    bap[:, oc : oc + 1].to_broadcast((P, TN)),
                    ALU.add,
                )
            return go

        def get_vaug(b):
            if b not in v_aug:
                v_aug[b] = big.tile(
                    [P, TT, H, HD + 1], BF16, tag="vaug", name="v_aug"
                )
                nc.vector.memset(v_aug[b][:, :, :, HD : HD + 1], 1.0)
            return v_aug[b]

        def q_v_part(b, mt, n0):
            # one 6-matmul chain: v rows mt*128.., out features n0..n0+nsz
            def go():
                get_vaug(b)
                nsz = min(TN, D - n0)
                pv = pmm.tile([P, TN], F32, tag="mm", name="pv")
                wv = wT["wv_w"]
                for cc in range(CC):
                    nc.tensor.matmul(
                        pv[:, :nsz],
                        xT[b][:, cc, mt * P : (mt + 1) * P],
                        wv[:, cc, n0 : n0 + nsz],
                        start=(cc == 0),
                        stop=(cc == CC - 1),
                    )
                h0 = n0 // HD
                nh = nsz // HD
                nc.vector.tensor_tensor(
                    v_aug[b][:, mt, h0 : h0 + nh, 0:HD],
                    pv[:, :nsz].rearrange("p (h d) -> p h d", d=HD),
                    bias_bc["wv_b"][:, n0 : n0 + nsz].rearrange(
                        "p (h d) -> p h d", d=HD
                    ),
                    ALU.add,
                )
            return go

        def q_recip(b, ic):
            def go():
                sl = rsum[b][:, ic * TN : (ic + 1) * TN]
                nc.vector.reciprocal_approx_fast(sl, sl)
                if b not in recip_r:
                    recip_r[b] = small.tile(
                        [H, S], BF16, tag="recip", name="recip_r"
                    )
                nc.vector.tensor_copy(recip_r[b][:, ic * TN : (ic + 1) * TN], sl)
            return go

        def q_passB(b, ic, hcs):
            # broadcast recip across partitions and normalize attn_T in place
            def go():
                for hc in hcs:
                    pb = pmm.tile([P, TN], F32, tag="mm", name="pb")
                    nc.tensor.matmul(
                        pb,
                        sel[:, hc * P : (hc + 1) * P],
                        recip_r[b][:, ic * TN : (ic + 1) * TN],
                        start=True,
                        stop=True,
                    )
                    sl = attn_T[b][:, hc, ic * TN : (ic + 1) * TN]
                    nc.vector.tensor_tensor(sl, sl, pb, ALU.mult)
            return go

        def q_out_proj(b, mt):
            def go():
                out_sb = iop.tile([P, D], F32, tag="osb", name="out_sb", bufs=2)
                wo = wT["wo_w"]
                for n0 in range(0, D, TN):
                    nsz = min(TN, D - n0)
                    pf = pmm.tile([P, TN], F32, tag="mm", name="pf")
                    for cc in range(CC):
                        nc.tensor.matmul(
                            pf[:, :nsz],
                            attn_T[b][:, cc, mt * P : (mt + 1) * P],
                            wo[:, cc, n0 : n0 + nsz],
                            start=(cc == 0),
                            stop=(cc == CC - 1),
                        )
                    nc.vector.tensor_tensor(
                        out_sb[:, n0 : n0 + nsz],
                        pf[:, :nsz],
                        bias_bc["wo_b"][:, n0 : n0 + nsz],
                        ALU.add,
                    )
                nc.sync.dma_start(out_d[b, mt * P : (mt + 1) * P, :], out_sb)
            return go

        # ---------------- pair-loop building blocks ----------------
        class Phase:
            __slots__ = ("b", "p", "ic", "probs", "po")

            def __init__(self, b, p, ic):
                self.b, self.p, self.ic = b, p, ic
                self.probs = [None] * JT
                self.po = None

        def emit_qk_exp(ph, jt):
            b, p, ic = ph.b, ph.p, ph.ic
            qc = qk_tiles[("q", b, p)]
            kc = qk_tiles[("k", b, p)]
            sq = psq.tile([P, 2, TN], F32, tag="sq", name="sq")
            for hi in range(2):
                hp = hi * HD
                nc.tensor.matmul(
                    sq[:, hi],
                    kc[hp : hp + HD, jt * P : (jt + 1) * P],
                    qc[hp : hp + HD, ic * TN : (ic + 1) * TN],
                    start=True,
                    stop=True,
                )
            pt = pbt.tile([P, 2, TN], BF16, tag="pbt", name="probsT")
            nc.scalar.activation(
                pt, sq, AF.Exp, bias=ln16, scale=float(1.0 / np.sqrt(HD))
            )
            ph.probs[jt] = pt

        def emit_pv_step(ph, jt):
            if ph.po is None:
                ph.po = [
                    ppv.tile([HD + 1, TN], F32, tag="pv", name=f"po{hi}")
                    for hi in range(2)
                ]
            for hi in range(2):
                h = ph.p * 2 + hi
                nc.tensor.matmul(
                    ph.po[hi],
                    v_aug[ph.b][:, jt, h, :],
                    ph.probs[jt][:, hi, :],
                    start=(jt == 0),
                    stop=(jt == JT - 1),
                )

        def emit_pv_evac(ph):
            b, p, ic = ph.b, ph.p, ph.ic
            if b not in attn_T:
                attn_T[b] = big.tile([P, CC, S], BF16, tag="attnT", name="attn_T")
            if b not in rsum:
                rsum[b] = small.tile([H, S], F32, tag="rsum", name="rsum")
            isl = slice(ic * TN, (ic + 1) * TN)
            # even head: rows 0:64 land on partitions 0:64 directly
            nc.vector.tensor_copy(attn_T[b][0:HD, p, isl], ph.po[0][:HD, :])
            # odd head: DVE lanes can't cross partitions; bounce via DMA
            tmp = small.tile([HD, TN], BF16, tag="odd", name="odd_tmp", bufs=3)
            nc.vector.tensor_copy(tmp, ph.po[1][:HD, :])
            nc.gpsimd.dma_start(attn_T[b][HD:P, p, isl], tmp)
            # denominators: row 64 of each -> staging -> rsum rows 2p, 2p+1
            for hi in range(2):
                st = st64p.tile([P, TN], F32, tag="st64", name="st64")
                nc.vector.tensor_copy(st[HD : HD + 1, :], ph.po[hi][HD : HD + 1, :])
                nc.gpsimd.dma_start(
                    rsum[b][2 * p + hi : 2 * p + hi + 1, isl],
                    st[HD : HD + 1, :],
                )

        # ---------------- schedule ----------------
        AUX = deque()

        def pump(n=1):
            for _ in range(n):
                if AUX:
                    AUX.popleft()()

        # startup, ordered for shortest path to the first QK matmul:
        # x tt0-3 -> wq oc0 -> wk oc0 -> chunk0 halves (q,nt0), (k,nt0)
        for tt in range(4):
            q_x_dma(0, tt)()
            q_x_transpose(0, tt)()
        q_w_transpose("wq_w", 0)()
        q_w_transpose("wk_w", 0)()
        q_qk_chunk_part(0, 0, "q", 0)()
        q_qk_chunk_part(0, 0, "k", 0)()

        # everything else the first two pairs need, in deadline order
        for tt in range(4, TT):
            AUX.append(q_x_dma(0, tt))
            AUX.append(q_x_transpose(0, tt))
        AUX.append(q_qk_chunk_part(0, 0, "k", 1))
        AUX.append(q_qk_chunk_part(0, 0, "q", 1))
        # v heads 0-7 (used from pair 0 on) need wv columns 0:512 = oc 0-3
        for oc in range(4):
            AUX.append(q_w_transpose("wv_w", oc))
        for mt in range(TT):
            AUX.append(q_v_part(0, mt, 0))
        for oc in range(4, CC):
            AUX.append(q_w_transpose("wv_w", oc))
        AUX.append(q_w_transpose("wq_w", 1))
        AUX.append(q_w_transpose("wk_w", 1))
        for nt in range(NT):
            AUX.append(q_qk_chunk_part(0, 1, "k", nt))
            AUX.append(q_qk_chunk_part(0, 1, "q", nt))
        for mt in range(TT):
            AUX.append(q_v_part(0, mt, TN))
        for oc in range(2, CC):
            AUX.append(q_w_transpose("wq_w", oc))
            AUX.append(q_w_transpose("wk_w", oc))

        pending = None
        for k in range(2 * 6):
            b, p = divmod(k, 6)
            # aux deadline scheduling for this pair-slot
            if p <= 3:
                for nt in range(NT):
                    AUX.append(q_qk_chunk_part(b, p + 2, "k", nt))
                    AUX.append(q_qk_chunk_part(b, p + 2, "q", nt))
            if k == 2:
                for tt in range(TT):
                    AUX.append(q_x_dma(1, tt))
                    AUX.append(q_x_transpose(1, tt))
            if k == 3:
                for mt in range(TT):
                    AUX.append(q_v_part(1, mt, 0))
            if k == 4:
                for mt in range(TT):
                    AUX.append(q_v_part(1, mt, TN))
                # wo overwrites wv's buffer after the last V-proj read
                for oc in range(CC):
                    AUX.append(q_w_transpose("wo_w", oc))
                for nt in range(NT):
                    AUX.append(q_qk_chunk_part(1, 0, "k", nt))
                    AUX.append(q_qk_chunk_part(1, 0, "q", nt))
            if k == 5:
                for nt in range(NT):
                    AUX.append(q_qk_chunk_part(1, 1, "k", nt))
                    AUX.append(q_qk_chunk_part(1, 1, "q", nt))

            for ic in range(NT):
                ph = Phase(b, p, ic)
                for jt in range(JT):
                    if pending is not None:
                        emit_pv_step(pending, jt)
                    emit_qk_exp(ph, jt)
                    # bootstrap phase has no P@V yet: pump harder so the
                    # v projection is fully emitted before its first read
                    pump(1 if pending is not None else 3)
                if pending is not None:
                    emit_pv_evac(pending)
                    if pending.p == 5:
                        # last pair of batch done for this ic: normalize
                        # and emit that ic's output-projection tiles
                        pb_, ic_ = pending.b, pending.ic
                        AUX.append(q_recip(pb_, ic_))
                        AUX.append(q_passB(pb_, ic_, range(0, 3)))
                        AUX.append(q_passB(pb_, ic_, range(3, CC)))
                        for mt in range(4 * ic_, 4 * (ic_ + 1)):
                            AUX.append(q_out_proj(pb_, mt))
                pending = ph

        # drain: final phase's P@V, evac, tail normalization + projection
        for jt in range(JT):
            emit_pv_step(pending, jt)
            pump(1)
        emit_pv_evac(pending)
        AUX.append(q_recip(1, 1))
        AUX.append(q_passB(1, 1, range(0, 3)))
        AUX.append(q_passB(1, 1, range(3, CC)))
        for mt in range(4, TT):
            AUX.append(q_out_proj(1, mt))
        while AUX:
            pump(1)


_BUILD_LOCK = threading.Lock()
_BUILT = {}


def build():
    with _BUILD_LOCK:
        if "nc" in _BUILT:
            return _BUILT["nc"]
        nc = bacc.Bacc(
            "TRN2",
            target_bir_lowering=False,
            debug=False,
            enable_asserts=True,
            num_devices=N_CORES,
        )
        ins = {
            "x": nc.dram_tensor("x", [BPC, S, D], F32, kind="ExternalInput").ap(),
            "sel": nc.dram_tensor(
                "sel", [H, H * HD], F32, kind="ExternalInput"
            ).ap(),
        }
        for w in ("wq_w", "wk_w", "wv_w", "wo_w"):
            ins[w] = nc.dram_tensor(w, [D, D], F32, kind="ExternalInput").ap()
        for bn in ("wq_b", "wk_b", "wv_b", "wo_b"):
            ins[bn] = nc.dram_tensor(bn, [D], F32, kind="ExternalInput").ap()
        outs = {
            "out": nc.dram_tensor(
                "out", [BPC, S, D], F32, kind="ExternalOutput"
            ).ap()
        }
        with tile.TileContext(nc) as tc:
            build_kernel(tc, outs, ins)
        nc.compile()
        _BUILT["nc"] = nc
        return nc


def make_in_maps(inputs):
    x = np.ascontiguousarray(np.asarray(inputs["x"], dtype=np.float32))
    shared = {
        k: np.ascontiguousarray(np.asarray(inputs[k], dtype=np.float32))
        for k in (
            "wq_w", "wq_b", "wk_w", "wk_b", "wv_w", "wv_b", "wo_w", "wo_b",
        )
    }
    sel = np.kron(np.eye(H, dtype=np.float32), np.ones((1, HD), np.float32))
    in_maps = []
    for c in range(N_CORES):
        m = {"x": x[c * BPC : (c + 1) * BPC], "sel": sel}
        m.update(shared)
        in_maps.append(m)
    return in_maps


def _ensure_profile_hook():
    """Install the axon NTFF profile hook shim if the container lacks it."""
    try:
        from antenv.axon_hooks import get_axon_ntff_profile_hook  # noqa: F401

        return
    except ImportError:
        pass
    try:
        import sys
        import types

        from trn_agent_boot.trn_boot import _ntff_profile_via_ctypes

        state = {"h": None}
        mod = types.ModuleType("antenv.axon_hooks")
        mod.set_axon_ntff_profile_hook = lambda h: state.__setitem__("h", h)
        mod.get_axon_ntff_profile_hook = lambda: state["h"]
        sys.modules["antenv.axon_hooks"] = mod
        mod.set_axon_ntff_profile_hook(
            _ntff_profile_via_ctypes("/opt/axon/libaxon_pjrt.so")
        )

        import concourse.bass_utils as bu

        orig_upload = bu.upload_artifacts

        def _safe_upload(d, *a, **k):
            try:
                return orig_upload(d, *a, **k)
            except Exception:
                return str(d)

        bu.upload_artifacts = _safe_upload
    except Exception:
        pass


def run(inputs, trace=False, **kwargs):
    """Returns (full_output [B,S,D] f32, BassKernelResults)."""
    if trace:
        _ensure_profile_hook()
    nc = build()
    res = run_bass_kernel_spmd(
        nc, make_in_maps(inputs), core_ids=list(range(N_CORES)),
        trace=trace, **kwargs,
    )
    out = np.concatenate([res.results[c]["out"] for c in range(N_CORES)], axis=0)
    return out, res


def kernel(**inputs):
    try:
        out, _ = run(inputs, trace=False)
    except Exception:
        # transient device hiccups (e.g. a prior crashed session) recover
        # on retry; the graph is already built/compiled at this point
        out, _ = run(inputs, trace=False)
    return out
